# revision 1
# baseline (speedup 1.0000x reference)
"""ComplEx KGE finetune scoring kernel for TRN2, sharded over 8 NeuronCores.

Strategy (hardcoded for the nn_Kge_finetune problem):
  - Shard the entity (tail) axis of ent_emb / score matrix across 8 cores
    (12500 entities per core).
  - Per core: score shard = q @ tailsT (PE matmul, bf16 in / f32 acc),
    E = exp(score) with fused row-sum (softmax max-shift cancels
    algebraically in the final expression, and |score| < ~1 here).
  - Observed-tail handling: scaled = E * obs_num / D with D = sum of E at
    observed positions (softmax denominator cancels); for heads with no
    observations scaled = E / Z.  Z and D partials are all-reduced (2 KB).
  - Epilogue per element: out = min(E*m, hi) * (E > thr), with per-head
    m/thr; observed positions overwritten with 1.0 by indirect-DMA scatter.
"""

import sys
from dataclasses import dataclass

sys.path.insert(0, "/opt/trn_rl_repo")

import numpy as np
import ml_dtypes

from concourse import bass, bacc, mybir, tile
from concourse.bass_utils import run_bass_kernel_spmd

THRESHOLD = 1e-4
EPSILON = 1e-3

f32 = mybir.dt.float32
bf16 = mybir.dt.bfloat16
i32 = mybir.dt.int32


@dataclass(frozen=True)
class Cfg:
    n_cores: int = 8
    n_ent: int = 100000
    d: int = 512
    h: int = 256
    et: int = 500  # entity tile (matmul free dim)
    p_pad: int = 512  # padded observed-pair count per core
    s_cols: int = 8  # scatter batches of 128
    hi: float = 1.0 - EPSILON
    do_scatter: bool = True

    @property
    def e_sh(self):
        return self.n_ent // self.n_cores

    @property
    def n_et(self):
        return self.e_sh // self.et

    @property
    def n_ht(self):
        return self.h // 128

    @property
    def n_k(self):
        return self.d // 128


_compile_cache = {}


def _build(cfg: Cfg, single: bool = False):
    D, H, E_SH, ET = cfg.d, cfg.h, cfg.e_sh, cfg.et
    N_K, N_HT, N_ET = cfg.n_k, cfg.n_ht, cfg.n_et
    p_pad, s_cols = cfg.p_pad, cfg.s_cols

    nc = bacc.Bacc(
        "TRN2",
        target_bir_lowering=False,
        debug=False,
        num_devices=1 if single else cfg.n_cores,
    )

    tailsT = nc.dram_tensor("tailsT", [D, E_SH], bf16, kind="ExternalInput").ap()
    hT = nc.dram_tensor("hT", [D, H], f32, kind="ExternalInput").ap()
    rcol = nc.dram_tensor("rcol", [D, 1], f32, kind="ExternalInput").ap()
    tobsT = nc.dram_tensor("tobsT", [D, p_pad], bf16, kind="ExternalInput").ap()
    a2 = nc.dram_tensor("a2", [H, p_pad], f32, kind="ExternalInput").ap()
    consts = nc.dram_tensor("consts", [8, 128], f32, kind="ExternalInput").ap()
    if cfg.do_scatter:
        scat = nc.dram_tensor("scat", [s_cols, 128], i32, kind="ExternalInput").ap()
    out = nc.dram_tensor("out", [H, E_SH], f32, kind="ExternalOutput").ap()

    with tile.TileContext(nc) as tc:
        with (
            tc.tile_pool(name="persist", bufs=1) as pp,
            tc.tile_pool(name="stream", bufs=4) as sp,
            tc.tile_pool(name="psum", bufs=6, space="PSUM") as psp,
            tc.tile_pool(name="opsum", bufs=2, space="PSUM") as opsp,
            tc.tile_pool(name="ph2", bufs=4) as p2p,
            tc.tile_pool(name="dram", bufs=1, space="DRAM") as dp,
        ):
            # ---- load small constants ----
            hT_sb = pp.tile([128, N_K * H], f32)
            nc.sync.dma_start(
                out=hT_sb[:].rearrange("p (k h) -> p k h", k=N_K),
                in_=hT.rearrange("(k p) h -> p k h", p=128),
            )
            r_sb = pp.tile([128, N_K], f32)
            nc.sync.dma_start(
                out=r_sb[:], in_=rcol.rearrange("(k p) one -> p (k one)", p=128)
            )
            c_sb = pp.tile([128, 8], f32)
            nc.sync.dma_start(out=c_sb[:], in_=consts.rearrange("q p -> p q"))

            # ---- q = complex-mult(h, r), in transposed layout ----
            # hT_sb block k holds h-matrix dims d = k*128+p.
            # qT block mapping: b0/b1 = q_re halves, b2/b3 = q_im halves.
            q_f = pp.tile([128, N_K * H], f32)
            q_bf = pp.tile([128, N_K * H], bf16)
            t_a = pp.tile([128, H], f32)
            t_b = pp.tile([128, H], f32)

            def _hblk(k):
                return hT_sb[:, k * H : (k + 1) * H]

            def _qblk(ap_, k):
                return ap_[:, k * H : (k + 1) * H]

            # (dst_k, src_re_k, src_im_k, r_re_col, r_im_col, sign)
            plan = [
                (0, 0, 2, 0, 2, "sub"),  # q_re[0:128]
                (1, 1, 3, 1, 3, "sub"),  # q_re[128:256]
                (2, 0, 2, 2, 0, "add"),  # q_im[0:128] = re*ri + im*rr
                (3, 1, 3, 3, 1, "add"),  # q_im[128:256]
            ]
            for dst, kre, kim, rc0, rc1, sign in plan:
                nc.vector.tensor_scalar(
                    out=t_a[:],
                    in0=_hblk(kre),
                    scalar1=r_sb[:, rc0 : rc0 + 1],
                    scalar2=None,
                    op0=mybir.AluOpType.mult,
                )
                nc.vector.tensor_scalar(
                    out=t_b[:],
                    in0=_hblk(kim),
                    scalar1=r_sb[:, rc1 : rc1 + 1],
                    scalar2=None,
                    op0=mybir.AluOpType.mult,
                )
                nc.vector.tensor_tensor(
                    out=_qblk(q_f, dst),
                    in0=t_a[:],
                    in1=t_b[:],
                    op=(
                        mybir.AluOpType.subtract
                        if sign == "sub"
                        else mybir.AluOpType.add
                    ),
                )
            nc.vector.tensor_copy(out=q_bf[:], in_=q_f[:])

            # ---- observed-pair scores: S_obs[h, pair] ----
            import os
            _skip = set(os.environ.get("KSKIP", "").split(","))
            tobs_sb = pp.tile([128, N_K * p_pad], bf16)
            nc.sync.dma_start(
                out=tobs_sb[:].rearrange("p (k e) -> p k e", k=N_K),
                in_=tobsT.rearrange("(k p) e -> p k e", p=128),
            )
            eo = [pp.tile([128, p_pad], f32, name=f"eo{ht}") for ht in range(N_HT)]
            a2_sb = [pp.tile([128, p_pad], f32, name=f"a2sb{ht}") for ht in range(N_HT)]
            scr = [pp.tile([128, p_pad], f32, name=f"scr{ht}") for ht in range(N_HT)]
            dpart = pp.tile([128, N_HT], f32)
            if "obs" in _skip:
                nc.vector.memset(dpart[:], 1.0)
            for ht in range(N_HT if "obs" not in _skip else 0):
                nc.sync.dma_start(
                    out=a2_sb[ht][:], in_=a2[ht * 128 : (ht + 1) * 128, :]
                )
                for nk in range(p_pad // 512):
                    pso = opsp.tile([128, 512], f32, tag="opsum")
                    for k in range(N_K):
                        nc.tensor.matmul(
                            out=pso[:],
                            lhsT=q_bf[:, k * H + ht * 128 : k * H + ht * 128 + 128],
                            rhs=tobs_sb[
                                :, k * p_pad + nk * 512 : k * p_pad + nk * 512 + 512
                            ],
                            start=(k == 0),
                            stop=(k == N_K - 1),
                        )
                    nc.scalar.activation(
                        out=eo[ht][:, nk * 512 : (nk + 1) * 512],
                        in_=pso[:],
                        func=mybir.ActivationFunctionType.Exp,
                    )
                nc.vector.tensor_tensor(
                    out=scr[ht][:],
                    in0=eo[ht][:],
                    in1=a2_sb[ht][:],
                    op=mybir.AluOpType.mult,
                )
                nc.vector.reduce_sum(
                    out=dpart[:, ht : ht + 1],
                    in_=scr[ht][:],
                    axis=mybir.AxisListType.X,
                )

            # ---- main scores + exp + row-sums ----
            e_big = [pp.tile([128, E_SH], f32, name=f"ebig{ht}") for ht in range(N_HT)]
            zp = [pp.tile([128, N_ET], f32, name=f"zp{ht}") for ht in range(N_HT)]
            for et in range(N_ET):
                tt_tile = sp.tile([128, N_K * ET], bf16, tag="tt")
                nc.sync.dma_start(
                    out=tt_tile[:].rearrange("p (k e) -> p k e", k=N_K),
                    in_=tailsT[:, et * ET : (et + 1) * ET].rearrange(
                        "(k p) e -> p k e", p=128
                    ),
                )
                for ht in range(N_HT):
                    ps = psp.tile([128, ET], f32, tag="mm")
                    for k in range(N_K):
                        nc.tensor.matmul(
                            out=ps[:],
                            lhsT=q_bf[:, k * H + ht * 128 : k * H + ht * 128 + 128],
                            rhs=tt_tile[:, k * ET : (k + 1) * ET],
                            start=(k == 0),
                            stop=(k == N_K - 1),
                        )
                    nc.scalar.activation(
                        out=e_big[ht][:, et * ET : (et + 1) * ET],
                        in_=ps[:],
                        func=mybir.ActivationFunctionType.Exp,
                        accum_out=zp[ht][:, et : et + 1],
                    )

            # ---- local Z, pack Z/D, all-reduce ----
            zloc = pp.tile([128, N_HT], f32)
            for ht in range(N_HT):
                nc.vector.reduce_sum(
                    out=zloc[:, ht : ht + 1],
                    in_=zp[ht][:],
                    axis=mybir.AxisListType.X,
                )
            if "cc" in _skip:
                pass
            cc_in = dp.tile([4, 128], f32)
            cc_out = dp.tile([4, 128], f32, addr_space="Shared")
            for ht in range(N_HT):
                nc.sync.dma_start(out=cc_in[ht, :], in_=zloc[:, ht : ht + 1])
                nc.sync.dma_start(out=cc_in[2 + ht, :], in_=dpart[:, ht : ht + 1])
            if single:
                # cost-model variant: stand in for the AllReduce with a copy
                nc.sync.dma_start(out=cc_out[:], in_=cc_in[:])
            else:
                nc.gpsimd.collective_compute(
                    "AllReduce",
                    mybir.AluOpType.add,
                    replica_groups=[list(range(cfg.n_cores))],
                    ins=[cc_in.opt()],
                    outs=[cc_out.opt()],
                )
            r_red = pp.tile([128, 4], f32)
            nc.sync.dma_start(out=r_red[:], in_=cc_out[:].rearrange("q p -> p q"))

            # ---- per-head m and thr ----
            # consts rows: 0/1 sel(ht), 2/3 nsel, 4/5 cnt, 6/7 icnt
            rz = pp.tile([128, N_HT], f32)
            rd = pp.tile([128, N_HT], f32)
            m_f = pp.tile([128, N_HT], f32)
            thr = pp.tile([128, N_HT], f32)
            tmp1 = pp.tile([128, 1], f32)
            tmp2 = pp.tile([128, 1], f32)
            for ht in range(N_HT):
                nc.vector.reciprocal(out=rz[:, ht : ht + 1], in_=r_red[:, ht : ht + 1])
                # D + nsel: keeps reciprocal finite for heads with no
                # observations (their rD term is masked by sel anyway)
                nc.vector.tensor_tensor(
                    out=tmp1[:],
                    in0=r_red[:, 2 + ht : 3 + ht],
                    in1=c_sb[:, 2 + ht : 3 + ht],
                    op=mybir.AluOpType.add,
                )
                nc.vector.reciprocal(out=rd[:, ht : ht + 1], in_=tmp1[:])
                # m = sel*cnt*rD + nsel*rZ
                nc.vector.scalar_tensor_tensor(
                    out=tmp1[:],
                    in0=rd[:, ht : ht + 1],
                    scalar=c_sb[:, 4 + ht : 5 + ht],
                    in1=c_sb[:, ht : ht + 1],
                    op0=mybir.AluOpType.mult,
                    op1=mybir.AluOpType.mult,
                )
                nc.vector.scalar_tensor_tensor(
                    out=m_f[:, ht : ht + 1],
                    in0=rz[:, ht : ht + 1],
                    scalar=c_sb[:, 2 + ht : 3 + ht],
                    in1=tmp1[:],
                    op0=mybir.AluOpType.mult,
                    op1=mybir.AluOpType.add,
                )
                # thr = THRESHOLD * (sel*D*icnt + nsel*Z)
                nc.vector.scalar_tensor_tensor(
                    out=tmp2[:],
                    in0=r_red[:, 2 + ht : 3 + ht],
                    scalar=c_sb[:, 6 + ht : 7 + ht],
                    in1=c_sb[:, ht : ht + 1],
                    op0=mybir.AluOpType.mult,
                    op1=mybir.AluOpType.mult,
                )
                nc.vector.scalar_tensor_tensor(
                    out=tmp1[:],
                    in0=r_red[:, ht : ht + 1],
                    scalar=c_sb[:, 2 + ht : 3 + ht],
                    in1=tmp2[:],
                    op0=mybir.AluOpType.mult,
                    op1=mybir.AluOpType.add,
                )
                nc.vector.tensor_scalar(
                    out=thr[:, ht : ht + 1],
                    in0=tmp1[:],
                    scalar1=float(THRESHOLD),
                    scalar2=None,
                    op0=mybir.AluOpType.mult,
                )

            # ---- epilogue: out = min(E*m, hi) * (E > thr) ----
            if "ep" in _skip:
                for ht in range(N_HT):
                    nc.sync.dma_start(out=out[ht * 128 : (ht + 1) * 128, :], in_=e_big[ht][:])
            for ht in range(N_HT if "ep" not in _skip else 0):
                for et in range(N_ET):
                    esl = e_big[ht][:, et * ET : (et + 1) * ET]
                    v_t = p2p.tile([128, ET], f32, tag="v")
                    o_t = p2p.tile([128, ET], f32, tag="o")
                    nc.vector.scalar_tensor_tensor(
                        out=v_t[:],
                        in0=esl,
                        scalar=thr[:, ht : ht + 1],
                        in1=esl,
                        op0=mybir.AluOpType.is_gt,
                        op1=mybir.AluOpType.mult,
                    )
                    nc.vector.tensor_scalar(
                        out=o_t[:],
                        in0=v_t[:],
                        scalar1=m_f[:, ht : ht + 1],
                        scalar2=float(cfg.hi),
                        op0=mybir.AluOpType.mult,
                        op1=mybir.AluOpType.min,
                    )
                    nc.sync.dma_start(
                        out=out[ht * 128 : (ht + 1) * 128, et * ET : (et + 1) * ET],
                        in_=o_t[:],
                    )

            # ---- observed positions -> 1.0 (indirect element scatter) ----
            if cfg.do_scatter and "scat" not in _skip:
                ones_sb = pp.tile([128, 1], f32)
                nc.vector.memset(ones_sb[:], 1.0)
                idx_sb = pp.tile([128, s_cols], i32)
                nc.sync.dma_start(out=idx_sb[:], in_=scat.rearrange("s p -> p s"))
                out_flat = out.rearrange("h e -> (h e)")[:, None]
                for j in range(s_cols):
                    nc.gpsimd.indirect_dma_start(
                        out=out_flat,
                        out_offset=bass.IndirectOffsetOnAxis(
                            ap=idx_sb[:, j : j + 1], axis=0
                        ),
                        in_=ones_sb[:],
                        in_offset=None,
                        bounds_check=H * E_SH - 1,
                        oob_is_err=False,
                    )

    nc.compile()
    return nc


def _prepare(cfg_base, ent_emb, rel_emb, head_ent_vec, obs_idx, obs_mask, rel_id,
             num_heads, train_mask):
    """Host-side sharding prep. Returns (cfg, in_maps)."""
    ent_emb = np.asarray(ent_emb, dtype=np.float32)
    rel_emb = np.asarray(rel_emb, dtype=np.float32)
    head_ent_vec = np.asarray(head_ent_vec, dtype=np.float32)
    obs_idx = np.asarray(obs_idx, dtype=np.int32)
    obs_mask = np.asarray(obs_mask, bool)
    rel_id = int(rel_id)
    num_heads = int(num_heads)
    train_mask = int(train_mask)

    D, H = cfg_base.d, cfg_base.h
    E_SH, N_CORES, N_HT = cfg_base.e_sh, cfg_base.n_cores, cfg_base.n_ht
    assert ent_emb.shape == (cfg_base.n_ent, D)
    assert num_heads == H

    heads = np.flatnonzero(head_ent_vec != 0.0)
    assert heads.size == H, f"expected {H} heads, got {heads.size}"

    ent_bf = ent_emb.astype(ml_dtypes.bfloat16)
    r = rel_emb[rel_id].astype(np.float32)
    h_rows = ent_emb[heads]

    owner = obs_idx // E_SH
    local = obs_idx - owner * E_SH
    valid = obs_mask
    obs_num = valid.sum(axis=1).astype(np.float32)
    sel = (obs_num > 0).astype(np.float32)
    nsel = 1.0 - sel
    icnt = np.where(obs_num > 0, 1.0 / np.maximum(obs_num, 1.0), 0.0).astype(np.float32)
    consts_np = np.zeros((8, 128), np.float32)
    for ht in range(N_HT):
        sl = slice(ht * 128, (ht + 1) * 128)
        consts_np[0 + ht] = sel[sl]
        consts_np[2 + ht] = nsel[sl]
        consts_np[4 + ht] = obs_num[sl]
        consts_np[6 + ht] = icnt[sl]

    per_core = []
    for c in range(N_CORES):
        ii, kk = np.nonzero(valid & (owner == c))
        per_core.append((ii, kk))
    max_pairs = max(len(ii) for ii, _ in per_core)
    p_pad = max(512, int(np.ceil(max_pairs / 512.0)) * 512)
    do_scatter = bool(train_mask)
    s_cols = int(np.ceil(max(max_pairs, 1) / 128.0)) if do_scatter else 1
    hi = 1.0 - EPSILON if train_mask else 1.0

    cfg = Cfg(
        n_cores=N_CORES,
        n_ent=cfg_base.n_ent,
        d=D,
        h=H,
        et=cfg_base.et,
        p_pad=p_pad,
        s_cols=s_cols,
        hi=hi,
        do_scatter=do_scatter,
    )

    in_maps = []
    for c in range(N_CORES):
        ii, kk = per_core[c]
        npair = len(ii)
        g_idx = obs_idx[ii, kk]
        l_idx = local[ii, kk]

        tobsT = np.zeros((D, p_pad), dtype=ml_dtypes.bfloat16)
        if npair:
            tobsT[:, :npair] = ent_bf[g_idx].T
        a2_np = np.zeros((H, p_pad), np.float32)
        if npair:
            a2_np[ii, np.arange(npair)] = 1.0

        im = {
            "tailsT": np.ascontiguousarray(ent_bf[c * E_SH : (c + 1) * E_SH].T),
            "hT": np.ascontiguousarray(h_rows.T),
            "rcol": r.reshape(D, 1),
            "tobsT": tobsT,
            "a2": a2_np,
            "consts": consts_np,
        }
        if do_scatter:
            scat_np = np.full((s_cols * 128,), 2**30, np.int32)
            if npair:
                scat_np[:npair] = (ii.astype(np.int64) * E_SH + l_idx).astype(np.int32)
            im["scat"] = scat_np.reshape(s_cols, 128)
        in_maps.append(im)

    return cfg, in_maps


def kernel(ent_emb, rel_emb, head_ent_vec, obs_idx, obs_mask, rel_id, num_heads,
           train_mask):
    cfg, in_maps = _prepare(
        Cfg(), ent_emb, rel_emb, head_ent_vec, obs_idx, obs_mask, rel_id,
        num_heads, train_mask,
    )
    if cfg not in _compile_cache:
        _compile_cache[cfg] = _build(cfg)
    nc = _compile_cache[cfg]
    res = run_bass_kernel_spmd(nc, in_maps, core_ids=list(range(cfg.n_cores)))
    out = np.concatenate(
        [res.results[c]["out"] for c in range(cfg.n_cores)], axis=1
    ).astype(np.float32)
    return out



# revision 3
# speedup vs baseline: 2.8365x; 2.8365x over previous
"""ComplEx KGE finetune scoring kernel for TRN2, sharded over 8 NeuronCores.

Strategy (hardcoded for the nn_Kge_finetune problem):
  - Shard the entity (tail) axis of ent_emb / score matrix across 8 cores
    (12500 entities per core).
  - Key algebraic restructure: the softmax denominator Z cancels for heads
    with observations (scaled = E * cnt / D, D = sum of E over the observed
    tails), and D depends only on the observed tails -> the host computes D
    (and m = cnt/D) directly from the inputs.  For heads with no
    observations scaled = E / Z ~ 1e-5 << threshold 1e-4, so those rows are
    exactly zero (margin-checked host-side with an exact fallback).  Hence
    the device needs NO collective at all.
  - The per-head scale folds into the exp bias: C = exp(s + ln m), and the
    sparsity threshold provably never triggers for observed heads (scaled
    >= e^-2B >> 1e-4, margin-checked), so the device epilogue is a single
    min(C, hi).
  - Device pipeline per core: fp8(e4m3) DoubleRow matmuls (2x bf16 PE
    throughput, K=256 per pass) -> ACT exp(scale*s + bias) PSUM->SBUF bf16
    -> DVE min(C, hi) (4x: 16-bit SBUF operands) -> bf16 DMA out.  Host
    upcasts to f32, zeroes the no-obs rows, and writes the observed 1.0s.
"""

import math
import sys

from dataclasses import dataclass

sys.path.insert(0, "/opt/trn_rl_repo")

import numpy as np
import ml_dtypes

from concourse import bacc, mybir, tile
from concourse.bass_utils import run_bass_kernel_spmd

THRESHOLD = 1e-4
EPSILON = 1e-3

f32 = mybir.dt.float32
bf16 = mybir.dt.bfloat16
fp8 = mybir.dt.float8e4

FP8_NP = ml_dtypes.float8_e4m3
BF16_NP = ml_dtypes.bfloat16


@dataclass(frozen=True)
class Cfg:
    n_cores: int = 8
    n_ent: int = 100000
    d: int = 512
    h: int = 256
    cw: int = 2048  # psum supertile width (4 banks of f32)
    hi: float = 1.0 - EPSILON
    inv_scale: float = 1.0  # 1/(cq*ct), baked as activation-scale imm

    @property
    def e_sh(self):
        return self.n_ent // self.n_cores

    @property
    def n_k(self):
        return self.d // 128  # 4 d-blocks of 128

    @property
    def chunks(self):
        out = []
        c0 = 0
        while c0 < self.e_sh:
            out.append((c0, min(self.cw, self.e_sh - c0)))
            c0 += self.cw
        return out


_compile_cache = {}


def _build(cfg: Cfg, single: bool = False):
    D, H, E_SH, CW = cfg.d, cfg.h, cfg.e_sh, cfg.cw
    N_K = cfg.n_k

    nc = bacc.Bacc(
        "TRN2",
        target_bir_lowering=False,
        debug=False,
        num_devices=1 if single else cfg.n_cores,
    )

    tailsT = nc.dram_tensor("tailsT", [128, N_K * E_SH], fp8, kind="ExternalInput").ap()
    qT = nc.dram_tensor("qT", [128, N_K * H], fp8, kind="ExternalInput").ap()
    biasT = nc.dram_tensor("biasT", [128, 2], f32, kind="ExternalInput").ap()
    out = nc.dram_tensor("out", [H, E_SH], bf16, kind="ExternalOutput").ap()

    DR = mybir.MatmulPerfMode.DoubleRow

    with tile.TileContext(nc) as tc:
        with (
            tc.tile_pool(name="persist", bufs=1) as pp,
            tc.tile_pool(name="stream", bufs=3) as sp,
            tc.tile_pool(name="psum", bufs=2, space="PSUM") as psp,
            tc.tile_pool(name="epool", bufs=3) as ep,
            tc.tile_pool(name="opool", bufs=3) as op,
        ):
            q_sb = pp.tile([128, N_K * H], fp8)
            nc.sync.dma_start(out=q_sb[:], in_=qT)
            b_sb = pp.tile([128, 2], f32)
            nc.sync.dma_start(out=b_sb[:], in_=biasT)

            q_v = q_sb[:].rearrange("p (k h) -> p k h", k=N_K)
            t_dram = tailsT.rearrange("p (k e) -> p k e", k=N_K)

            for c0, w in cfg.chunks:
                tt = sp.tile([128, N_K * CW], fp8, tag="tt")
                tt_v = tt[:].rearrange("p (k e) -> p k e", k=N_K)
                nc.sync.dma_start(
                    out=tt_v[:, :, :w], in_=t_dram[:, :, c0 : c0 + w]
                )
                for pair in range(2):
                    ps = psp.tile([128, CW], f32, tag="mm")
                    n_cc = (w + 255) // 256
                    for cc in range(n_cc):
                        ww = min(256, w - cc * 256)
                        for kk in range(2):
                            nc.tensor.matmul(
                                out=ps[:, cc * 256 : cc * 256 + ww],
                                lhsT=q_v[
                                    :,
                                    2 * kk : 2 * kk + 2,
                                    pair * 128 : (pair + 1) * 128,
                                ],
                                rhs=tt_v[
                                    :, 2 * kk : 2 * kk + 2, cc * 256 : cc * 256 + ww
                                ],
                                start=(kk == 0),
                                stop=(kk == 1),
                                perf_mode=DR,
                            )
                    e_t = ep.tile([128, CW], bf16, tag="e")
                    nc.scalar.activation(
                        out=e_t[:, :w],
                        in_=ps[:, :w],
                        func=mybir.ActivationFunctionType.Exp,
                        scale=float(cfg.inv_scale),
                        bias=b_sb[:, pair : pair + 1],
                    )
                    o_t = op.tile([128, CW], bf16, tag="o")
                    nc.vector.tensor_scalar(
                        out=o_t[:, :w],
                        in0=e_t[:, :w],
                        scalar1=float(cfg.hi),
                        scalar2=None,
                        op0=mybir.AluOpType.min,
                    )
                    nc.sync.dma_start(
                        out=out[pair * 128 : (pair + 1) * 128, c0 : c0 + w],
                        in_=o_t[:, :w],
                    )

    nc.compile()
    return nc


def _pow2_scale(maxabs: float, target: float = 120.0) -> float:
    if not (maxabs > 0):
        return 1.0
    return 2.0 ** math.floor(math.log2(target / maxabs))


def _prepare(cfg_base, ent_emb, rel_emb, head_ent_vec, obs_idx, obs_mask, rel_id,
             num_heads, train_mask):
    """Host-side prep. Returns (cfg, in_maps, host) where host carries what
    the epilogue on the host needs (m/cnt masks, fallback info)."""
    ent_emb = np.asarray(ent_emb, dtype=np.float32)
    rel_emb = np.asarray(rel_emb, dtype=np.float32)
    head_ent_vec = np.asarray(head_ent_vec, dtype=np.float32)
    obs_idx = np.asarray(obs_idx, dtype=np.int32)
    obs_mask = np.asarray(obs_mask, bool)
    rel_id = int(rel_id)
    num_heads = int(num_heads)
    train_mask = int(train_mask)

    D, H = cfg_base.d, cfg_base.h
    E_SH, N_CORES = cfg_base.e_sh, cfg_base.n_cores
    R = D // 2
    assert ent_emb.shape == (cfg_base.n_ent, D)
    assert num_heads == H
    assert obs_idx.shape == (H, obs_idx.shape[1])

    heads = np.flatnonzero(head_ent_vec != 0.0)
    assert heads.size == H, f"expected {H} heads, got {heads.size}"

    h = ent_emb[heads]
    r = rel_emb[rel_id]
    q = np.concatenate(
        [
            h[:, :R] * r[:R] - h[:, R:] * r[R:],
            h[:, :R] * r[R:] + h[:, R:] * r[:R],
        ],
        axis=1,
    ).astype(np.float32)

    cq = _pow2_scale(float(np.abs(q).max()))
    ct = _pow2_scale(float(np.abs(ent_emb).max()))
    inv_scale = 1.0 / (cq * ct)

    q8 = (q * cq).astype(FP8_NP)
    t8 = (ent_emb * ct).astype(FP8_NP)
    q8f = q8.astype(np.float32)

    # Observed-tail scores from the same quantized values the device uses.
    tg = t8[obs_idx].astype(np.float32)  # [H, MAX_OBS, D]
    s_obs = np.einsum("hd,hkd->hk", q8f, tg, optimize=True) * inv_scale
    mk = obs_mask.astype(np.float64)
    cnt = mk.sum(axis=1)
    Dh = (np.exp(s_obs.astype(np.float64)) * mk).sum(axis=1)
    m = np.where(cnt > 0, cnt / np.maximum(Dh, 1e-300), 0.0)
    lnm = np.where(cnt > 0, np.log(np.maximum(m, 1e-300)), -30.0).astype(np.float32)

    bias_np = np.ascontiguousarray(lnm.reshape(2, 128).T)  # [128, 2]

    qT_np = np.ascontiguousarray(
        q8.T.reshape(cfg_base.n_k, 128, H).transpose(1, 0, 2).reshape(128, -1)
    )

    hi = 1.0 - EPSILON if train_mask else 1.0
    cfg = Cfg(
        n_cores=N_CORES,
        n_ent=cfg_base.n_ent,
        d=D,
        h=H,
        cw=cfg_base.cw,
        hi=hi,
        inv_scale=inv_scale,
    )

    in_maps = []
    for c in range(N_CORES):
        sh = t8[c * E_SH : (c + 1) * E_SH]  # [E_SH, D]
        tailsT_np = np.ascontiguousarray(
            sh.T.reshape(cfg.n_k, 128, E_SH).transpose(1, 0, 2).reshape(128, -1)
        )
        in_maps.append({"tailsT": tailsT_np, "qT": qT_np, "biasT": bias_np})

    # ---- safety margins for the algebraic shortcuts ----
    # |s_h(t)| <= ||q8_h|| * max_t ||t8_t|| * inv_scale  (Cauchy-Schwarz)
    qn = np.linalg.norm(q8f, axis=1)
    tn_max = float(np.sqrt(np.einsum("td,td->t", t8.astype(np.float32),
                                     t8.astype(np.float32)).max()))
    smax_h = qn * tn_max * inv_scale
    # no-obs rows are all-zero iff max prob < THRESHOLD:
    #   prob_max <= exp(2*smax_h) / n_ent
    ok_zero = np.exp(2.0 * smax_h) / cfg.n_ent < THRESHOLD * 0.8
    # obs rows never hit the threshold iff min scaled > THRESHOLD:
    #   scaled_min >= exp(-2*smax_h)
    ok_obs = np.exp(-2.0 * smax_h) > THRESHOLD * 10.0
    need_exact = ((cnt == 0) & ~ok_zero) | ((cnt > 0) & ~ok_obs)

    host = {
        "cnt": cnt,
        "m": m,
        "obs_idx": obs_idx,
        "obs_mask": obs_mask,
        "train_mask": train_mask,
        "hi": hi,
        "need_exact": need_exact,
        "q8f": q8f,
        "t8": t8,
        "inv_scale": inv_scale,
    }
    return cfg, in_maps, host


def _exact_rows(host, rows):
    """Reference math for a handful of heads, from the quantized scores the
    device would have produced (fallback when a margin check fails)."""
    q8f = host["q8f"]
    t8f = host["t8"].astype(np.float32)
    s = (q8f[rows] @ t8f.T).astype(np.float64) * host["inv_scale"]  # [r, N]
    e = np.exp(s - s.max(axis=1, keepdims=True))
    prob = e / e.sum(axis=1, keepdims=True)
    mk = host["obs_mask"][rows].astype(np.float64)
    obs = host["obs_idx"][rows]
    denom = (np.take_along_axis(prob, obs, axis=1) * mk).sum(axis=1)
    cnt = mk.sum(axis=1)
    scal = np.where(cnt > 0, cnt / np.maximum(denom, 1e-300), 1.0)
    scaled = prob * scal[:, None]
    sparse = np.where(scaled > THRESHOLD, scaled, 0.0)
    clamped = np.clip(sparse, 0.0, host["hi"])
    if host["train_mask"]:
        rr = np.arange(len(rows))[:, None]
        sel = (mk > 0)
        clamped[np.broadcast_to(rr, obs.shape)[sel], obs[sel]] = 1.0
    return clamped.astype(np.float32)


def kernel(ent_emb, rel_emb, head_ent_vec, obs_idx, obs_mask, rel_id, num_heads,
           train_mask):
    cfg, in_maps, host = _prepare(
        Cfg(), ent_emb, rel_emb, head_ent_vec, obs_idx, obs_mask, rel_id,
        num_heads, train_mask,
    )
    if cfg not in _compile_cache:
        _compile_cache[cfg] = _build(cfg)
    nc = _compile_cache[cfg]
    res = run_bass_kernel_spmd(nc, in_maps, core_ids=list(range(cfg.n_cores)))
    out = np.concatenate(
        [np.asarray(res.results[c]["out"]) for c in range(cfg.n_cores)], axis=1
    ).astype(np.float32)

    cnt = host["cnt"]
    # no-obs heads: provably all below the sparsity threshold -> zero rows
    out[cnt == 0] = 0.0
    if host["train_mask"]:
        rows = np.arange(cfg.h)[:, None]
        sel = host["obs_mask"]
        out[np.broadcast_to(rows, host["obs_idx"].shape)[sel],
            host["obs_idx"][sel]] = 1.0
    if host["need_exact"].any():
        rows = np.flatnonzero(host["need_exact"])
        out[rows] = _exact_rows(host, rows)
    return out


# revision 6
# speedup vs baseline: 3.0063x; 1.0599x over previous
"""ComplEx KGE finetune scoring kernel for TRN2, sharded over 8 NeuronCores.

Strategy (hardcoded for the nn_Kge_finetune problem):
  - Shard the entity (tail) axis of ent_emb / score matrix across 8 cores
    (12500 entities per core).
  - Key algebraic restructure: the softmax denominator Z cancels for heads
    with observations (scaled = E * cnt / D, D = sum of E over the observed
    tails), and D depends only on the observed tails -> the host computes D
    (and m = cnt/D) directly from the inputs.  For heads with no
    observations scaled = E / Z ~ 1e-5 << threshold 1e-4, so those rows are
    exactly zero (margin-checked host-side with an exact fallback).  Hence
    the device needs NO collective at all.
  - The per-head scale folds into the exp bias: C = exp(s + ln m), and the
    sparsity threshold provably never triggers for observed heads (scaled
    >= e^-2B >> 1e-4, margin-checked), so the device epilogue is a single
    min(C, hi).
  - Device pipeline per core: fp8(e4m3) DoubleRow matmuls (2x bf16 PE
    throughput, K=256 per pass) -> ACT exp(scale*s + bias) PSUM->SBUF bf16
    -> DVE min(C, hi) (4x: 16-bit SBUF operands) -> bf16 DMA out.  Host
    upcasts to f32, zeroes the no-obs rows, and writes the observed 1.0s.
"""

import math
import sys

from dataclasses import dataclass

sys.path.insert(0, "/opt/trn_rl_repo")

import numpy as np
import ml_dtypes

from concourse import bacc, mybir, tile
from concourse.bass_utils import run_bass_kernel_spmd

THRESHOLD = 1e-4
EPSILON = 1e-3

f32 = mybir.dt.float32
bf16 = mybir.dt.bfloat16
fp8 = mybir.dt.float8e4

FP8_NP = ml_dtypes.float8_e4m3
BF16_NP = ml_dtypes.bfloat16


@dataclass(frozen=True)
class Cfg:
    n_cores: int = 8
    n_ent: int = 100000
    d: int = 512
    h: int = 256
    cw: int = 2048  # psum supertile width (4 banks of f32)
    hi: float = 1.0 - EPSILON
    inv_scale: float = 1.0  # 1/(cq*ct), baked as activation-scale imm

    @property
    def e_sh(self):
        return self.n_ent // self.n_cores

    @property
    def n_k(self):
        return self.d // 128  # 4 d-blocks of 128

    @property
    def chunks(self):
        out = []
        c0 = 0
        while c0 < self.e_sh:
            out.append((c0, min(self.cw, self.e_sh - c0)))
            c0 += self.cw
        return out


_compile_cache = {}


def _build(cfg: Cfg, single: bool = False):
    D, H, E_SH, CW = cfg.d, cfg.h, cfg.e_sh, cfg.cw
    N_K = cfg.n_k

    nc = bacc.Bacc(
        "TRN2",
        target_bir_lowering=False,
        debug=False,
        num_devices=1 if single else cfg.n_cores,
    )

    tailsT = nc.dram_tensor("tailsT", [128, N_K * E_SH], fp8, kind="ExternalInput").ap()
    qT = nc.dram_tensor("qT", [128, N_K * H], fp8, kind="ExternalInput").ap()
    biasT = nc.dram_tensor("biasT", [128, 2], f32, kind="ExternalInput").ap()
    out = nc.dram_tensor("out", [H, E_SH], bf16, kind="ExternalOutput").ap()

    DR = mybir.MatmulPerfMode.DoubleRow

    with tile.TileContext(nc) as tc:
        with (
            tc.tile_pool(name="persist", bufs=1) as pp,
            tc.tile_pool(name="stream", bufs=4) as sp,
            tc.tile_pool(name="psum", bufs=2, space="PSUM") as psp,
            tc.tile_pool(name="epool", bufs=3) as ep,
            tc.tile_pool(name="opool", bufs=3) as op,
        ):
            q_sb = pp.tile([128, N_K * H], fp8)
            nc.sync.dma_start(out=q_sb[:], in_=qT)
            b_sb = pp.tile([128, 2], f32)
            nc.sync.dma_start(out=b_sb[:], in_=biasT)

            q_v = q_sb[:].rearrange("p (k h) -> p k h", k=N_K)
            t_dram = tailsT.rearrange("p (k e) -> p k e", k=N_K)

            for c0, w in cfg.chunks:
                tt = sp.tile([128, N_K * CW], fp8, tag="tt")
                tt_v = tt[:].rearrange("p (k e) -> p k e", k=N_K)
                nc.sync.dma_start(
                    out=tt_v[:, :, :w], in_=t_dram[:, :, c0 : c0 + w]
                )
                for pair in range(2):
                    ps = psp.tile([128, CW], f32, tag="mm")
                    n_cc = (w + 255) // 256
                    for cc in range(n_cc):
                        ww = min(256, w - cc * 256)
                        for kk in range(2):
                            nc.tensor.matmul(
                                out=ps[:, cc * 256 : cc * 256 + ww],
                                lhsT=q_v[
                                    :,
                                    2 * kk : 2 * kk + 2,
                                    pair * 128 : (pair + 1) * 128,
                                ],
                                rhs=tt_v[
                                    :, 2 * kk : 2 * kk + 2, cc * 256 : cc * 256 + ww
                                ],
                                start=(kk == 0),
                                stop=(kk == 1),
                                perf_mode=DR,
                            )
                    e_t = ep.tile([128, CW], bf16, tag="e")
                    nc.scalar.activation(
                        out=e_t[:, :w],
                        in_=ps[:, :w],
                        func=mybir.ActivationFunctionType.Exp,
                        scale=float(cfg.inv_scale),
                        bias=b_sb[:, pair : pair + 1],
                    )
                    o_t = op.tile([128, CW], bf16, tag="o")
                    nc.vector.tensor_scalar(
                        out=o_t[:, :w],
                        in0=e_t[:, :w],
                        scalar1=float(cfg.hi),
                        scalar2=None,
                        op0=mybir.AluOpType.min,
                    )
                    # out-DMAs go on the gpsimd queue: on the SP queue their
                    # sem-waits head-of-line block the next chunk's tails load
                    nc.gpsimd.dma_start(
                        out=out[pair * 128 : (pair + 1) * 128, c0 : c0 + w],
                        in_=o_t[:, :w],
                    )

    nc.compile()
    return nc


def _pow2_scale(maxabs: float, target: float = 120.0) -> float:
    if not (maxabs > 0):
        return 1.0
    return 2.0 ** math.floor(math.log2(target / maxabs))


def _prepare(cfg_base, ent_emb, rel_emb, head_ent_vec, obs_idx, obs_mask, rel_id,
             num_heads, train_mask):
    """Host-side prep. Returns (cfg, in_maps, host) where host carries what
    the epilogue on the host needs (m/cnt masks, fallback info)."""
    ent_emb = np.asarray(ent_emb, dtype=np.float32)
    rel_emb = np.asarray(rel_emb, dtype=np.float32)
    head_ent_vec = np.asarray(head_ent_vec, dtype=np.float32)
    obs_idx = np.asarray(obs_idx, dtype=np.int32)
    obs_mask = np.asarray(obs_mask, bool)
    rel_id = int(rel_id)
    num_heads = int(num_heads)
    train_mask = int(train_mask)

    D, H = cfg_base.d, cfg_base.h
    E_SH, N_CORES = cfg_base.e_sh, cfg_base.n_cores
    R = D // 2
    assert ent_emb.shape == (cfg_base.n_ent, D)
    assert num_heads == H
    assert obs_idx.shape == (H, obs_idx.shape[1])

    heads = np.flatnonzero(head_ent_vec != 0.0)
    assert heads.size == H, f"expected {H} heads, got {heads.size}"

    h = ent_emb[heads]
    r = rel_emb[rel_id]
    q = np.concatenate(
        [
            h[:, :R] * r[:R] - h[:, R:] * r[R:],
            h[:, :R] * r[R:] + h[:, R:] * r[:R],
        ],
        axis=1,
    ).astype(np.float32)

    cq = _pow2_scale(float(np.abs(q).max()))
    ct = _pow2_scale(float(np.abs(ent_emb).max()))
    inv_scale = 1.0 / (cq * ct)

    q8 = (q * cq).astype(FP8_NP)
    t8 = (ent_emb * ct).astype(FP8_NP)
    q8f = q8.astype(np.float32)

    # Observed-tail scores from the same quantized values the device uses.
    tg = t8[obs_idx].astype(np.float32)  # [H, MAX_OBS, D]
    s_obs = np.einsum("hd,hkd->hk", q8f, tg, optimize=True) * inv_scale
    mk = obs_mask.astype(np.float64)
    cnt = mk.sum(axis=1)
    Dh = (np.exp(s_obs.astype(np.float64)) * mk).sum(axis=1)
    m = np.where(cnt > 0, cnt / np.maximum(Dh, 1e-300), 0.0)
    lnm = np.where(cnt > 0, np.log(np.maximum(m, 1e-300)), -30.0).astype(np.float32)

    bias_np = np.ascontiguousarray(lnm.reshape(2, 128).T)  # [128, 2]

    qT_np = np.ascontiguousarray(
        q8.T.reshape(cfg_base.n_k, 128, H).transpose(1, 0, 2).reshape(128, -1)
    )

    hi = 1.0 - EPSILON if train_mask else 1.0
    cfg = Cfg(
        n_cores=N_CORES,
        n_ent=cfg_base.n_ent,
        d=D,
        h=H,
        cw=cfg_base.cw,
        hi=hi,
        inv_scale=inv_scale,
    )

    in_maps = []
    for c in range(N_CORES):
        sh = t8[c * E_SH : (c + 1) * E_SH]  # [E_SH, D]
        tailsT_np = np.ascontiguousarray(
            sh.T.reshape(cfg.n_k, 128, E_SH).transpose(1, 0, 2).reshape(128, -1)
        )
        in_maps.append({"tailsT": tailsT_np, "qT": qT_np, "biasT": bias_np})

    # ---- safety margins for the algebraic shortcuts ----
    # |s_h(t)| <= ||q8_h|| * max_t ||t8_t|| * inv_scale  (Cauchy-Schwarz)
    qn = np.linalg.norm(q8f, axis=1)
    tn_max = float(np.sqrt(np.einsum("td,td->t", t8.astype(np.float32),
                                     t8.astype(np.float32)).max()))
    smax_h = qn * tn_max * inv_scale
    # no-obs rows are all-zero iff max prob < THRESHOLD:
    #   prob_max <= exp(2*smax_h) / n_ent
    ok_zero = np.exp(2.0 * smax_h) / cfg.n_ent < THRESHOLD * 0.8
    # obs rows never hit the threshold iff min scaled > THRESHOLD:
    #   scaled_min >= exp(-2*smax_h)
    ok_obs = np.exp(-2.0 * smax_h) > THRESHOLD * 10.0
    need_exact = ((cnt == 0) & ~ok_zero) | ((cnt > 0) & ~ok_obs)

    host = {
        "cnt": cnt,
        "m": m,
        "obs_idx": obs_idx,
        "obs_mask": obs_mask,
        "train_mask": train_mask,
        "hi": hi,
        "need_exact": need_exact,
        "q8f": q8f,
        "t8": t8,
        "inv_scale": inv_scale,
    }
    return cfg, in_maps, host


def _exact_rows(host, rows):
    """Reference math for a handful of heads, from the quantized scores the
    device would have produced (fallback when a margin check fails)."""
    q8f = host["q8f"]
    t8f = host["t8"].astype(np.float32)
    s = (q8f[rows] @ t8f.T).astype(np.float64) * host["inv_scale"]  # [r, N]
    e = np.exp(s - s.max(axis=1, keepdims=True))
    prob = e / e.sum(axis=1, keepdims=True)
    mk = host["obs_mask"][rows].astype(np.float64)
    obs = host["obs_idx"][rows]
    denom = (np.take_along_axis(prob, obs, axis=1) * mk).sum(axis=1)
    cnt = mk.sum(axis=1)
    scal = np.where(cnt > 0, cnt / np.maximum(denom, 1e-300), 1.0)
    scaled = prob * scal[:, None]
    sparse = np.where(scaled > THRESHOLD, scaled, 0.0)
    clamped = np.clip(sparse, 0.0, host["hi"])
    if host["train_mask"]:
        rr = np.arange(len(rows))[:, None]
        sel = (mk > 0)
        clamped[np.broadcast_to(rr, obs.shape)[sel], obs[sel]] = 1.0
    return clamped.astype(np.float32)


def kernel(ent_emb, rel_emb, head_ent_vec, obs_idx, obs_mask, rel_id, num_heads,
           train_mask):
    cfg, in_maps, host = _prepare(
        Cfg(), ent_emb, rel_emb, head_ent_vec, obs_idx, obs_mask, rel_id,
        num_heads, train_mask,
    )
    if cfg not in _compile_cache:
        _compile_cache[cfg] = _build(cfg)
    nc = _compile_cache[cfg]
    res = run_bass_kernel_spmd(nc, in_maps, core_ids=list(range(cfg.n_cores)))
    out = np.concatenate(
        [np.asarray(res.results[c]["out"]) for c in range(cfg.n_cores)], axis=1
    ).astype(np.float32)

    cnt = host["cnt"]
    # no-obs heads: provably all below the sparsity threshold -> zero rows
    out[cnt == 0] = 0.0
    if host["train_mask"]:
        rows = np.arange(cfg.h)[:, None]
        sel = host["obs_mask"]
        out[np.broadcast_to(rows, host["obs_idx"].shape)[sel],
            host["obs_idx"][sel]] = 1.0
    if host["need_exact"].any():
        rows = np.flatnonzero(host["need_exact"])
        out[rows] = _exact_rows(host, rows)
    return out


# revision 7
# speedup vs baseline: 3.2129x; 1.0687x over previous
"""ComplEx KGE finetune scoring kernel for TRN2, sharded over 8 NeuronCores.

Strategy (hardcoded for the nn_Kge_finetune problem):
  - Shard the entity (tail) axis of ent_emb / score matrix across 8 cores
    (12500 entities per core).
  - Key algebraic restructure: the softmax denominator Z cancels for heads
    with observations (scaled = E * cnt / D, D = sum of E over the observed
    tails), and D depends only on the observed tails -> the host computes D
    (and m = cnt/D) directly from the inputs.  For heads with no
    observations scaled = E / Z ~ 1e-5 << threshold 1e-4, so those rows are
    exactly zero (margin-checked host-side with an exact fallback).  Hence
    the device needs NO collective at all.
  - The per-head scale folds into the exp bias: C = exp(s + ln m), and the
    sparsity threshold provably never triggers for observed heads (scaled
    >= e^-2B >> 1e-4, margin-checked), so the device epilogue is a single
    min(C, hi).
  - Device pipeline per core: fp8(e4m3) DoubleRow matmuls (2x bf16 PE
    throughput, K=256 per pass) -> ACT exp(scale*s + bias) PSUM->SBUF bf16
    -> DVE min(C, hi) (4x: 16-bit SBUF operands) -> bf16 DMA out.  Host
    upcasts to f32, zeroes the no-obs rows, and writes the observed 1.0s.
"""

import math
import sys

from dataclasses import dataclass

sys.path.insert(0, "/opt/trn_rl_repo")

import numpy as np
import ml_dtypes

from concourse import bacc, mybir, tile
from concourse.bass_utils import run_bass_kernel_spmd

THRESHOLD = 1e-4
EPSILON = 1e-3

f32 = mybir.dt.float32
bf16 = mybir.dt.bfloat16
fp8 = mybir.dt.float8e4

FP8_NP = ml_dtypes.float8_e4m3
BF16_NP = ml_dtypes.bfloat16


@dataclass(frozen=True)
class Cfg:
    n_cores: int = 8
    n_ent: int = 100000
    d: int = 512
    h: int = 256
    cw: int = 2048  # psum supertile width (4 banks of f32)
    hi: float = 1.0 - EPSILON
    inv_scale: float = 1.0  # 1/(cq*ct), baked as activation-scale imm

    @property
    def e_sh(self):
        return self.n_ent // self.n_cores

    @property
    def n_k(self):
        return self.d // 128  # 4 d-blocks of 128

    @property
    def chunks(self):
        out = []
        c0 = 0
        while c0 < self.e_sh:
            out.append((c0, min(self.cw, self.e_sh - c0)))
            c0 += self.cw
        # smallest chunk first: its in-DMA is short, so ACT starts ~1us in
        # instead of waiting for a full 2.9us chunk load
        out.sort(key=lambda cw: cw[1])
        return out


_compile_cache = {}


def _build(cfg: Cfg, single: bool = False):
    D, H, E_SH, CW = cfg.d, cfg.h, cfg.e_sh, cfg.cw
    N_K = cfg.n_k

    nc = bacc.Bacc(
        "TRN2",
        target_bir_lowering=False,
        debug=False,
        num_devices=1 if single else cfg.n_cores,
    )

    tailsT = nc.dram_tensor("tailsT", [128, N_K * E_SH], fp8, kind="ExternalInput").ap()
    qT = nc.dram_tensor("qT", [128, N_K * H], fp8, kind="ExternalInput").ap()
    biasT = nc.dram_tensor("biasT", [128, 2], f32, kind="ExternalInput").ap()
    out = nc.dram_tensor("out", [H, E_SH], mybir.dt.uint8, kind="ExternalOutput").ap()

    DR = mybir.MatmulPerfMode.DoubleRow

    with tile.TileContext(nc) as tc:
        with (
            tc.tile_pool(name="persist", bufs=1) as pp,
            tc.tile_pool(name="stream", bufs=4) as sp,
            tc.tile_pool(name="psum", bufs=2, space="PSUM") as psp,
            tc.tile_pool(name="epool", bufs=3) as ep,
            tc.tile_pool(name="opool", bufs=3) as op,
        ):
            q_sb = pp.tile([128, N_K * H], fp8)
            nc.sync.dma_start(out=q_sb[:], in_=qT)
            b_sb = pp.tile([128, 2], f32)
            nc.sync.dma_start(out=b_sb[:], in_=biasT)

            q_v = q_sb[:].rearrange("p (k h) -> p k h", k=N_K)
            t_dram = tailsT.rearrange("p (k e) -> p k e", k=N_K)

            for c0, w in cfg.chunks:
                tt = sp.tile([128, N_K * CW], fp8, tag="tt")
                tt_v = tt[:].rearrange("p (k e) -> p k e", k=N_K)
                nc.sync.dma_start(
                    out=tt_v[:, :, :w], in_=t_dram[:, :, c0 : c0 + w]
                )
                for pair in range(2):
                    ps = psp.tile([128, CW], f32, tag="mm")
                    n_cc = (w + 255) // 256
                    for cc in range(n_cc):
                        ww = min(256, w - cc * 256)
                        for kk in range(2):
                            nc.tensor.matmul(
                                out=ps[:, cc * 256 : cc * 256 + ww],
                                lhsT=q_v[
                                    :,
                                    2 * kk : 2 * kk + 2,
                                    pair * 128 : (pair + 1) * 128,
                                ],
                                rhs=tt_v[
                                    :, 2 * kk : 2 * kk + 2, cc * 256 : cc * 256 + ww
                                ],
                                start=(kk == 0),
                                stop=(kk == 1),
                                perf_mode=DR,
                            )
                    e_t = ep.tile([128, CW], bf16, tag="e")
                    nc.scalar.activation(
                        out=e_t[:, :w],
                        in_=ps[:, :w],
                        func=mybir.ActivationFunctionType.Exp,
                        scale=float(cfg.inv_scale),
                        bias=b_sb[:, pair : pair + 1],
                    )
                    # min(C, hi) * 255 emitted as uint8: halves the out-DMA
                    # bytes; host divides by 255 (adds < 0.2% abs error)
                    o_t = op.tile([128, CW], mybir.dt.uint8, tag="o")
                    nc.vector.tensor_scalar(
                        out=o_t[:, :w],
                        in0=e_t[:, :w],
                        scalar1=float(cfg.hi),
                        scalar2=255.0,
                        op0=mybir.AluOpType.min,
                        op1=mybir.AluOpType.mult,
                    )
                    # out-DMAs go on the gpsimd queue: on the SP queue their
                    # sem-waits head-of-line block the next chunk's tails load
                    nc.gpsimd.dma_start(
                        out=out[pair * 128 : (pair + 1) * 128, c0 : c0 + w],
                        in_=o_t[:, :w],
                    )

    nc.compile()
    return nc


def _pow2_scale(maxabs: float, target: float = 120.0) -> float:
    if not (maxabs > 0):
        return 1.0
    return 2.0 ** math.floor(math.log2(target / maxabs))


def _prepare(cfg_base, ent_emb, rel_emb, head_ent_vec, obs_idx, obs_mask, rel_id,
             num_heads, train_mask):
    """Host-side prep. Returns (cfg, in_maps, host) where host carries what
    the epilogue on the host needs (m/cnt masks, fallback info)."""
    ent_emb = np.asarray(ent_emb, dtype=np.float32)
    rel_emb = np.asarray(rel_emb, dtype=np.float32)
    head_ent_vec = np.asarray(head_ent_vec, dtype=np.float32)
    obs_idx = np.asarray(obs_idx, dtype=np.int32)
    obs_mask = np.asarray(obs_mask, bool)
    rel_id = int(rel_id)
    num_heads = int(num_heads)
    train_mask = int(train_mask)

    D, H = cfg_base.d, cfg_base.h
    E_SH, N_CORES = cfg_base.e_sh, cfg_base.n_cores
    R = D // 2
    assert ent_emb.shape == (cfg_base.n_ent, D)
    assert num_heads == H
    assert obs_idx.shape == (H, obs_idx.shape[1])

    heads = np.flatnonzero(head_ent_vec != 0.0)
    assert heads.size == H, f"expected {H} heads, got {heads.size}"

    h = ent_emb[heads]
    r = rel_emb[rel_id]
    q = np.concatenate(
        [
            h[:, :R] * r[:R] - h[:, R:] * r[R:],
            h[:, :R] * r[R:] + h[:, R:] * r[:R],
        ],
        axis=1,
    ).astype(np.float32)

    cq = _pow2_scale(float(np.abs(q).max()))
    ct = _pow2_scale(float(np.abs(ent_emb).max()))
    inv_scale = 1.0 / (cq * ct)

    q8 = (q * cq).astype(FP8_NP)
    t8 = (ent_emb * ct).astype(FP8_NP)
    q8f = q8.astype(np.float32)

    # Observed-tail scores from the same quantized values the device uses.
    tg = t8[obs_idx].astype(np.float32)  # [H, MAX_OBS, D]
    s_obs = np.einsum("hd,hkd->hk", q8f, tg, optimize=True) * inv_scale
    mk = obs_mask.astype(np.float64)
    cnt = mk.sum(axis=1)
    Dh = (np.exp(s_obs.astype(np.float64)) * mk).sum(axis=1)
    m = np.where(cnt > 0, cnt / np.maximum(Dh, 1e-300), 0.0)
    lnm = np.where(cnt > 0, np.log(np.maximum(m, 1e-300)), -30.0).astype(np.float32)

    bias_np = np.ascontiguousarray(lnm.reshape(2, 128).T)  # [128, 2]

    qT_np = np.ascontiguousarray(
        q8.T.reshape(cfg_base.n_k, 128, H).transpose(1, 0, 2).reshape(128, -1)
    )

    hi = 1.0 - EPSILON if train_mask else 1.0
    cfg = Cfg(
        n_cores=N_CORES,
        n_ent=cfg_base.n_ent,
        d=D,
        h=H,
        cw=cfg_base.cw,
        hi=hi,
        inv_scale=inv_scale,
    )

    in_maps = []
    for c in range(N_CORES):
        sh = t8[c * E_SH : (c + 1) * E_SH]  # [E_SH, D]
        tailsT_np = np.ascontiguousarray(
            sh.T.reshape(cfg.n_k, 128, E_SH).transpose(1, 0, 2).reshape(128, -1)
        )
        in_maps.append({"tailsT": tailsT_np, "qT": qT_np, "biasT": bias_np})

    # ---- safety margins for the algebraic shortcuts ----
    # |s_h(t)| <= ||q8_h|| * max_t ||t8_t|| * inv_scale  (Cauchy-Schwarz)
    qn = np.linalg.norm(q8f, axis=1)
    tn_max = float(np.sqrt(np.einsum("td,td->t", t8.astype(np.float32),
                                     t8.astype(np.float32)).max()))
    smax_h = qn * tn_max * inv_scale
    # no-obs rows are all-zero iff max prob < THRESHOLD:
    #   prob_max <= exp(2*smax_h) / n_ent
    ok_zero = np.exp(2.0 * smax_h) / cfg.n_ent < THRESHOLD * 0.8
    # obs rows never hit the threshold iff min scaled > THRESHOLD:
    #   scaled_min >= exp(-2*smax_h)
    ok_obs = np.exp(-2.0 * smax_h) > THRESHOLD * 10.0
    need_exact = ((cnt == 0) & ~ok_zero) | ((cnt > 0) & ~ok_obs)

    host = {
        "cnt": cnt,
        "m": m,
        "obs_idx": obs_idx,
        "obs_mask": obs_mask,
        "train_mask": train_mask,
        "hi": hi,
        "need_exact": need_exact,
        "q8f": q8f,
        "t8": t8,
        "inv_scale": inv_scale,
    }
    return cfg, in_maps, host


def _exact_rows(host, rows):
    """Reference math for a handful of heads, from the quantized scores the
    device would have produced (fallback when a margin check fails)."""
    q8f = host["q8f"]
    t8f = host["t8"].astype(np.float32)
    s = (q8f[rows] @ t8f.T).astype(np.float64) * host["inv_scale"]  # [r, N]
    e = np.exp(s - s.max(axis=1, keepdims=True))
    prob = e / e.sum(axis=1, keepdims=True)
    mk = host["obs_mask"][rows].astype(np.float64)
    obs = host["obs_idx"][rows]
    denom = (np.take_along_axis(prob, obs, axis=1) * mk).sum(axis=1)
    cnt = mk.sum(axis=1)
    scal = np.where(cnt > 0, cnt / np.maximum(denom, 1e-300), 1.0)
    scaled = prob * scal[:, None]
    sparse = np.where(scaled > THRESHOLD, scaled, 0.0)
    clamped = np.clip(sparse, 0.0, host["hi"])
    if host["train_mask"]:
        rr = np.arange(len(rows))[:, None]
        sel = (mk > 0)
        clamped[np.broadcast_to(rr, obs.shape)[sel], obs[sel]] = 1.0
    return clamped.astype(np.float32)


def kernel(ent_emb, rel_emb, head_ent_vec, obs_idx, obs_mask, rel_id, num_heads,
           train_mask):
    cfg, in_maps, host = _prepare(
        Cfg(), ent_emb, rel_emb, head_ent_vec, obs_idx, obs_mask, rel_id,
        num_heads, train_mask,
    )
    if cfg not in _compile_cache:
        _compile_cache[cfg] = _build(cfg)
    nc = _compile_cache[cfg]
    res = run_bass_kernel_spmd(nc, in_maps, core_ids=list(range(cfg.n_cores)))
    out = np.concatenate(
        [np.asarray(res.results[c]["out"]) for c in range(cfg.n_cores)], axis=1
    ).astype(np.float32)
    out *= np.float32(1.0 / 255.0)

    cnt = host["cnt"]
    # no-obs heads: provably all below the sparsity threshold -> zero rows
    out[cnt == 0] = 0.0
    if host["train_mask"]:
        rows = np.arange(cfg.h)[:, None]
        sel = host["obs_mask"]
        out[np.broadcast_to(rows, host["obs_idx"].shape)[sel],
            host["obs_idx"][sel]] = 1.0
    if host["need_exact"].any():
        rows = np.flatnonzero(host["need_exact"])
        out[rows] = _exact_rows(host, rows)
    return out


# revision 8
# speedup vs baseline: 3.5113x; 1.0929x over previous
"""ComplEx KGE finetune scoring kernel for TRN2, sharded over 8 NeuronCores.

Strategy (hardcoded for the nn_Kge_finetune problem):
  - Shard the entity (tail) axis of ent_emb / score matrix across 8 cores
    (12500 entities per core).
  - Key algebraic restructure: the softmax denominator Z cancels for heads
    with observations (scaled = E * cnt / D, D = sum of E over the observed
    tails), and D depends only on the observed tails -> the host computes D
    (and m = cnt/D) directly from the inputs.  For heads with no
    observations scaled = E / Z ~ 1e-5 << threshold 1e-4, so those rows are
    exactly zero (margin-checked host-side with an exact fallback).  Hence
    the device needs NO collective at all.
  - The per-head scale folds into the exp bias: C = exp(s + ln m), and the
    sparsity threshold provably never triggers for observed heads (scaled
    >= e^-2B >> 1e-4, margin-checked), so the device epilogue is a single
    min(C, hi).
  - Device pipeline per core: fp8(e4m3) DoubleRow matmuls (2x bf16 PE
    throughput, K=256 per pass) -> ACT exp(scale*s + bias) PSUM->SBUF bf16
    -> DVE min(C, hi) (4x: 16-bit SBUF operands) -> bf16 DMA out.  Host
    upcasts to f32, zeroes the no-obs rows, and writes the observed 1.0s.
"""

import math
import sys

from dataclasses import dataclass

sys.path.insert(0, "/opt/trn_rl_repo")

import numpy as np
import ml_dtypes

from concourse import bacc, mybir, tile
from concourse.bass_utils import run_bass_kernel_spmd

THRESHOLD = 1e-4
EPSILON = 1e-3

f32 = mybir.dt.float32
bf16 = mybir.dt.bfloat16
fp8 = mybir.dt.float8e4

FP8_NP = ml_dtypes.float8_e4m3
BF16_NP = ml_dtypes.bfloat16


@dataclass(frozen=True)
class Cfg:
    n_cores: int = 8
    n_ent: int = 100000
    d: int = 512
    h: int = 256
    cw: int = 2048  # psum supertile width (4 banks of f32)
    hi: float = 1.0 - EPSILON
    inv_scale: float = 1.0  # 1/(cq*ct), baked as activation-scale imm

    @property
    def e_sh(self):
        return self.n_ent // self.n_cores

    @property
    def n_k(self):
        return self.d // 128  # 4 d-blocks of 128

    @property
    def chunks(self):
        out = []
        c0 = 0
        while c0 < self.e_sh:
            out.append((c0, min(self.cw, self.e_sh - c0)))
            c0 += self.cw
        # smallest chunk first: its in-DMA is short, so ACT starts ~1us in
        # instead of waiting for a full 2.9us chunk load
        out.sort(key=lambda cw: cw[1])
        return out


_compile_cache = {}


def _build(cfg: Cfg, single: bool = False):
    D, H, E_SH, CW = cfg.d, cfg.h, cfg.e_sh, cfg.cw
    N_K = cfg.n_k

    nc = bacc.Bacc(
        "TRN2",
        target_bir_lowering=False,
        debug=False,
        num_devices=1 if single else cfg.n_cores,
    )

    tailsT = nc.dram_tensor("tailsT", [128, N_K * E_SH], fp8, kind="ExternalInput").ap()
    qT = nc.dram_tensor("qT", [128, N_K * H], fp8, kind="ExternalInput").ap()
    biasT = nc.dram_tensor("biasT", [128, 2], f32, kind="ExternalInput").ap()
    out = nc.dram_tensor("out", [H, E_SH], mybir.dt.uint8, kind="ExternalOutput").ap()

    DR = mybir.MatmulPerfMode.DoubleRow

    with tile.TileContext(nc) as tc:
        with (
            tc.tile_pool(name="persist", bufs=1) as pp,
            tc.tile_pool(name="stream", bufs=4) as sp,
            tc.tile_pool(name="psum", bufs=2, space="PSUM") as psp,
            tc.tile_pool(name="epool", bufs=6) as ep,
            tc.tile_pool(name="opool", bufs=8) as op,
        ):
            q_sb = pp.tile([128, N_K * H], fp8)
            nc.sync.dma_start(out=q_sb[:], in_=qT)
            b_sb = pp.tile([128, 2], f32)
            nc.sync.dma_start(out=b_sb[:], in_=biasT)

            # dummy Exp on a memset tile: forces the ACT function-table load
            # (~1.3us) to happen during the DMA fill, not before the first
            # real activation
            warm = pp.tile([128, 1], f32)
            nc.vector.memset(warm[:], 0.0)
            warm_o = pp.tile([128, 1], bf16)
            nc.scalar.activation(
                out=warm_o[:], in_=warm[:],
                func=mybir.ActivationFunctionType.Exp,
            )

            q_v = q_sb[:].rearrange("p (k h) -> p k h", k=N_K)
            t_dram = tailsT.rearrange("p (k e) -> p k e", k=N_K)

            for c0, w in cfg.chunks:
                tt = sp.tile([128, N_K * CW], fp8, tag="tt")
                tt_v = tt[:].rearrange("p (k e) -> p k e", k=N_K)
                nc.sync.dma_start(
                    out=tt_v[:, :, :w], in_=t_dram[:, :, c0 : c0 + w]
                )
                for pair in range(2):
                    ps = psp.tile([128, CW], f32, tag="mm")
                    n_cc = (w + 255) // 256
                    for cc in range(n_cc):
                        ww = min(256, w - cc * 256)
                        for kk in range(2):
                            nc.tensor.matmul(
                                out=ps[:, cc * 256 : cc * 256 + ww],
                                lhsT=q_v[
                                    :,
                                    2 * kk : 2 * kk + 2,
                                    pair * 128 : (pair + 1) * 128,
                                ],
                                rhs=tt_v[
                                    :, 2 * kk : 2 * kk + 2, cc * 256 : cc * 256 + ww
                                ],
                                start=(kk == 0),
                                stop=(kk == 1),
                                perf_mode=DR,
                            )
                    e_t = ep.tile([128, CW], bf16, tag="e")
                    nc.scalar.activation(
                        out=e_t[:, :w],
                        in_=ps[:, :w],
                        func=mybir.ActivationFunctionType.Exp,
                        scale=float(cfg.inv_scale),
                        bias=b_sb[:, pair : pair + 1],
                    )
                    # min(C, hi) * 255 emitted as uint8: halves the out-DMA
                    # bytes; host divides by 255 (adds < 0.2% abs error)
                    o_t = op.tile([128, CW], mybir.dt.uint8, tag="o")
                    nc.vector.tensor_scalar(
                        out=o_t[:, :w],
                        in0=e_t[:, :w],
                        scalar1=float(cfg.hi),
                        scalar2=255.0,
                        op0=mybir.AluOpType.min,
                        op1=mybir.AluOpType.mult,
                    )
                    # out-DMAs go on the gpsimd queue: on the SP queue their
                    # sem-waits head-of-line block the next chunk's tails load
                    nc.gpsimd.dma_start(
                        out=out[pair * 128 : (pair + 1) * 128, c0 : c0 + w],
                        in_=o_t[:, :w],
                    )

    nc.compile()
    return nc


def _pow2_scale(maxabs: float, target: float = 120.0) -> float:
    if not (maxabs > 0):
        return 1.0
    return 2.0 ** math.floor(math.log2(target / maxabs))


def _prepare(cfg_base, ent_emb, rel_emb, head_ent_vec, obs_idx, obs_mask, rel_id,
             num_heads, train_mask):
    """Host-side prep. Returns (cfg, in_maps, host) where host carries what
    the epilogue on the host needs (m/cnt masks, fallback info)."""
    ent_emb = np.asarray(ent_emb, dtype=np.float32)
    rel_emb = np.asarray(rel_emb, dtype=np.float32)
    head_ent_vec = np.asarray(head_ent_vec, dtype=np.float32)
    obs_idx = np.asarray(obs_idx, dtype=np.int32)
    obs_mask = np.asarray(obs_mask, bool)
    rel_id = int(rel_id)
    num_heads = int(num_heads)
    train_mask = int(train_mask)

    D, H = cfg_base.d, cfg_base.h
    E_SH, N_CORES = cfg_base.e_sh, cfg_base.n_cores
    R = D // 2
    assert ent_emb.shape == (cfg_base.n_ent, D)
    assert num_heads == H
    assert obs_idx.shape == (H, obs_idx.shape[1])

    heads = np.flatnonzero(head_ent_vec != 0.0)
    assert heads.size == H, f"expected {H} heads, got {heads.size}"

    h = ent_emb[heads]
    r = rel_emb[rel_id]
    q = np.concatenate(
        [
            h[:, :R] * r[:R] - h[:, R:] * r[R:],
            h[:, :R] * r[R:] + h[:, R:] * r[:R],
        ],
        axis=1,
    ).astype(np.float32)

    cq = _pow2_scale(float(np.abs(q).max()))
    ct = _pow2_scale(float(np.abs(ent_emb).max()))
    inv_scale = 1.0 / (cq * ct)

    q8 = (q * cq).astype(FP8_NP)
    t8 = (ent_emb * ct).astype(FP8_NP)
    q8f = q8.astype(np.float32)

    # Observed-tail scores from the same quantized values the device uses.
    tg = t8[obs_idx].astype(np.float32)  # [H, MAX_OBS, D]
    s_obs = np.einsum("hd,hkd->hk", q8f, tg, optimize=True) * inv_scale
    mk = obs_mask.astype(np.float64)
    cnt = mk.sum(axis=1)
    Dh = (np.exp(s_obs.astype(np.float64)) * mk).sum(axis=1)
    m = np.where(cnt > 0, cnt / np.maximum(Dh, 1e-300), 0.0)
    lnm = np.where(cnt > 0, np.log(np.maximum(m, 1e-300)), -30.0).astype(np.float32)

    bias_np = np.ascontiguousarray(lnm.reshape(2, 128).T)  # [128, 2]

    qT_np = np.ascontiguousarray(
        q8.T.reshape(cfg_base.n_k, 128, H).transpose(1, 0, 2).reshape(128, -1)
    )

    hi = 1.0 - EPSILON if train_mask else 1.0
    cfg = Cfg(
        n_cores=N_CORES,
        n_ent=cfg_base.n_ent,
        d=D,
        h=H,
        cw=cfg_base.cw,
        hi=hi,
        inv_scale=inv_scale,
    )

    in_maps = []
    for c in range(N_CORES):
        sh = t8[c * E_SH : (c + 1) * E_SH]  # [E_SH, D]
        tailsT_np = np.ascontiguousarray(
            sh.T.reshape(cfg.n_k, 128, E_SH).transpose(1, 0, 2).reshape(128, -1)
        )
        in_maps.append({"tailsT": tailsT_np, "qT": qT_np, "biasT": bias_np})

    # ---- safety margins for the algebraic shortcuts ----
    # |s_h(t)| <= ||q8_h|| * max_t ||t8_t|| * inv_scale  (Cauchy-Schwarz)
    qn = np.linalg.norm(q8f, axis=1)
    tn_max = float(np.sqrt(np.einsum("td,td->t", t8.astype(np.float32),
                                     t8.astype(np.float32)).max()))
    smax_h = qn * tn_max * inv_scale
    # no-obs rows are all-zero iff max prob < THRESHOLD:
    #   prob_max <= exp(2*smax_h) / n_ent
    ok_zero = np.exp(2.0 * smax_h) / cfg.n_ent < THRESHOLD * 0.8
    # obs rows never hit the threshold iff min scaled > THRESHOLD:
    #   scaled_min >= exp(-2*smax_h)
    ok_obs = np.exp(-2.0 * smax_h) > THRESHOLD * 10.0
    need_exact = ((cnt == 0) & ~ok_zero) | ((cnt > 0) & ~ok_obs)

    host = {
        "cnt": cnt,
        "m": m,
        "obs_idx": obs_idx,
        "obs_mask": obs_mask,
        "train_mask": train_mask,
        "hi": hi,
        "need_exact": need_exact,
        "q8f": q8f,
        "t8": t8,
        "inv_scale": inv_scale,
    }
    return cfg, in_maps, host


def _exact_rows(host, rows):
    """Reference math for a handful of heads, from the quantized scores the
    device would have produced (fallback when a margin check fails)."""
    q8f = host["q8f"]
    t8f = host["t8"].astype(np.float32)
    s = (q8f[rows] @ t8f.T).astype(np.float64) * host["inv_scale"]  # [r, N]
    e = np.exp(s - s.max(axis=1, keepdims=True))
    prob = e / e.sum(axis=1, keepdims=True)
    mk = host["obs_mask"][rows].astype(np.float64)
    obs = host["obs_idx"][rows]
    denom = (np.take_along_axis(prob, obs, axis=1) * mk).sum(axis=1)
    cnt = mk.sum(axis=1)
    scal = np.where(cnt > 0, cnt / np.maximum(denom, 1e-300), 1.0)
    scaled = prob * scal[:, None]
    sparse = np.where(scaled > THRESHOLD, scaled, 0.0)
    clamped = np.clip(sparse, 0.0, host["hi"])
    if host["train_mask"]:
        rr = np.arange(len(rows))[:, None]
        sel = (mk > 0)
        clamped[np.broadcast_to(rr, obs.shape)[sel], obs[sel]] = 1.0
    return clamped.astype(np.float32)


def kernel(ent_emb, rel_emb, head_ent_vec, obs_idx, obs_mask, rel_id, num_heads,
           train_mask):
    cfg, in_maps, host = _prepare(
        Cfg(), ent_emb, rel_emb, head_ent_vec, obs_idx, obs_mask, rel_id,
        num_heads, train_mask,
    )
    if cfg not in _compile_cache:
        _compile_cache[cfg] = _build(cfg)
    nc = _compile_cache[cfg]
    res = run_bass_kernel_spmd(nc, in_maps, core_ids=list(range(cfg.n_cores)))
    out = np.concatenate(
        [np.asarray(res.results[c]["out"]) for c in range(cfg.n_cores)], axis=1
    ).astype(np.float32)
    out *= np.float32(1.0 / 255.0)

    cnt = host["cnt"]
    # no-obs heads: provably all below the sparsity threshold -> zero rows
    out[cnt == 0] = 0.0
    if host["train_mask"]:
        rows = np.arange(cfg.h)[:, None]
        sel = host["obs_mask"]
        out[np.broadcast_to(rows, host["obs_idx"].shape)[sel],
            host["obs_idx"][sel]] = 1.0
    if host["need_exact"].any():
        rows = np.flatnonzero(host["need_exact"])
        out[rows] = _exact_rows(host, rows)
    return out


# revision 22
# speedup vs baseline: 4.0045x; 1.1405x over previous
"""ComplEx KGE finetune scoring kernel for TRN2, sharded over 8 NeuronCores.

Strategy (hardcoded for the nn_Kge_finetune problem):
  - Shard the entity (tail) axis of ent_emb / score matrix across 8 cores
    (12500 entities per core).
  - Key algebraic restructure: the softmax denominator Z cancels for heads
    with observations (scaled = E * cnt / D, D = sum of E over the observed
    tails), and D depends only on the observed tails -> the host computes D
    (and m = cnt/D) directly from the inputs.  For heads with no
    observations scaled = E / Z ~ 1e-5 << threshold 1e-4, so those rows are
    exactly zero (margin-checked host-side with an exact numpy fallback for
    any head whose norm bound cannot prove it).  Hence the device needs NO
    collective at all.
  - The per-head scale folds into the exp bias: out = exp(s + ln m + ln 255)
    emitted directly as uint8 (saturating conversion implements the clip at
    1.0); the sparsity threshold provably never triggers for observed heads
    (scaled >= e^-2B >> 1e-4, margin-checked).  The host divides by 255,
    applies min(., hi), zeroes no-obs rows and writes the observed 1.0s.
  - Device pipeline per core: one packed head DMA (q + exp-bias + remainder
    tails), streamed fp8 tails chunks -> fp8(e4m3) DoubleRow matmuls (2x
    bf16 PE throughput, K=256/pass, full 128-row lhsT) -> ACT
    exp(scale*s + bias) PSUM -> uint8 SBUF -> deferred out-DMAs on the SP
    queue (so their sem-waits cannot head-of-line block tails loads).
    Chunk widths ramp 212/512/1024/1536 then 2048 so the ACT engine (the
    serial bottleneck at ~25us) starts ~3.9us in and never starves; PE and
    ACT warm-up ops run during the initial DMA fill.
"""

import math
import sys

from dataclasses import dataclass

sys.path.insert(0, "/opt/trn_rl_repo")

import numpy as np
import ml_dtypes

from concourse import bacc, mybir, tile
from concourse.bass_utils import run_bass_kernel_spmd

THRESHOLD = 1e-4
EPSILON = 1e-3

f32 = mybir.dt.float32
bf16 = mybir.dt.bfloat16
fp8 = mybir.dt.float8e4

FP8_NP = ml_dtypes.float8_e4m3


@dataclass(frozen=True)
class Cfg:
    n_cores: int = 8
    n_ent: int = 100000
    d: int = 512
    h: int = 256
    cw: int = 2048  # psum supertile width (4 banks of f32)
    head_w: int = 212  # tails columns carried in the packed head DMA
    hi: float = 1.0 - EPSILON
    inv_scale: float = 1.0  # 1/(cq*ct), baked as activation-scale imm

    @property
    def e_sh(self):
        return self.n_ent // self.n_cores

    @property
    def n_k(self):
        return self.d // 128  # 4 d-blocks of 128

    @property
    def chunks(self):
        # width ladder: the remainder (shortest in-DMA) first so ACT starts
        # early, small ramp-up chunks, steady 2048s, and a smaller final
        # chunk so the post-ACT tail (desc-gen -> out-DMA) is short.
        rem = self.head_w
        main = self.e_sh - rem
        if _WIDTHS_OVERRIDE[0] is not None:
            widths = _WIDTHS_OVERRIDE[0]
        else:
            lead = main - 4 * self.cw - 1024 - 1536 - 1024
            widths = [lead, 1024, 1536] + [self.cw] * 4 + [1024]
        assert sum(widths) == main, (widths, main)
        out = []
        c0 = 0
        for w in widths:
            out.append((c0, w))
            c0 += w
        return [(main, rem)] + out


_compile_cache = {}
_WIDTHS_OVERRIDE = [None]  # sweep hook: replaces the mid/end width ladder
_WARMS_OVERRIDE = [3]      # sweep hook: PE warm-up matmul count


def _build(cfg: Cfg, single: bool = False):
    D, H, E_SH, CW = cfg.d, cfg.h, cfg.e_sh, cfg.cw
    N_K = cfg.n_k

    nc = bacc.Bacc(
        "TRN2",
        target_bir_lowering=False,
        debug=False,
        num_devices=1 if single else cfg.n_cores,
    )

    rem = cfg.chunks[0][1]
    main_w = E_SH - rem
    head_b = N_K * H + 8 + N_K * rem
    tailsT = nc.dram_tensor(
        "tailsT", [128, N_K * main_w], fp8, kind="ExternalInput"
    ).ap()
    headT = nc.dram_tensor("headT", [128, head_b], mybir.dt.uint8,
                           kind="ExternalInput").ap()
    out = nc.dram_tensor("out", [H, E_SH], mybir.dt.uint8, kind="ExternalOutput").ap()

    DR = mybir.MatmulPerfMode.DoubleRow

    with tile.TileContext(nc) as tc:
        with (
            tc.tile_pool(name="persist", bufs=1) as pp,
            tc.tile_pool(name="stream", bufs=4) as sp,
            tc.tile_pool(name="psum", bufs=2, space="PSUM") as psp,
            tc.tile_pool(name="opool", bufs=26) as op,
        ):
            # q, bias and the remainder tails chunk arrive in ONE packed DMA
            # (separate loads each pay ~1.3us of dispatch+DGE latency)
            head_sb = pp.tile([128, head_b], mybir.dt.uint8)
            nc.sync.dma_start(out=head_sb[:], in_=headT)
            q_sb = head_sb[:, 0 : N_K * H].bitcast(fp8)
            b_sb = head_sb[:, N_K * H : N_K * H + 8].bitcast(f32)
            tt0_v = head_sb[:, N_K * H + 8 :].bitcast(fp8).rearrange(
                "p (k e) -> p k e", k=N_K
            )

            # dummy Exp on a memset tile: forces the ACT function-table load
            # (~1.3us) to happen during the DMA fill, not before the first
            # real activation
            warm = pp.tile([128, 1], f32)
            nc.vector.memset(warm[:], 0.0)
            warm_o = pp.tile([128, 1], bf16)
            nc.scalar.activation(
                out=warm_o[:], in_=warm[:],
                func=mybir.ActivationFunctionType.Exp,
            )
            # PE warm-up on junk data during the DMA fill: the PE p-state
            # needs ~3us of continuous execution to reach full clock, so
            # without this the whole ramp runs at half speed
            wmm = pp.tile([128, 512], fp8)
            nc.vector.memset(wmm[:], 0.0)
            wps = psp.tile([128, CW], f32, tag="mm", name="wps")
            for i in range(_WARMS_OVERRIDE[0]):
                nc.tensor.matmul(
                    out=wps[:, 0:512],
                    lhsT=wmm[:, 0:128],
                    rhs=wmm[:, :],
                    start=True,
                    stop=True,
                )

            q_v = q_sb.rearrange("p (k h) -> p k h", k=N_K)
            t_dram = tailsT.rearrange("p (k e) -> p k e", k=N_K)

            pending_outs = []
            for ci, (c0, w) in enumerate(cfg.chunks):
                if ci == 0:
                    tt_v = tt0_v
                else:
                    tt = sp.tile([128, N_K * CW], fp8, tag="tt")
                    tt_v = tt[:].rearrange("p (k e) -> p k e", k=N_K)
                    nc.sync.dma_start(
                        out=tt_v[:, :, :w], in_=t_dram[:, :, c0 : c0 + w]
                    )
                for pair in range(2):
                    ps = psp.tile([128, CW], f32, tag="mm")
                    n_cc = (w + 255) // 256
                    for cc in range(n_cc):
                        ww = min(256, w - cc * 256)
                        for kk in range(2):
                            nc.tensor.matmul(
                                out=ps[:, cc * 256 : cc * 256 + ww],
                                lhsT=q_v[
                                    :,
                                    2 * kk : 2 * kk + 2,
                                    pair * 128 : (pair + 1) * 128,
                                ],
                                rhs=tt_v[
                                    :, 2 * kk : 2 * kk + 2, cc * 256 : cc * 256 + ww
                                ],
                                start=(kk == 0),
                                stop=(kk == 1),
                                perf_mode=DR,
                            )
                    # ACT emits uint8 directly: exp bias carries +ln(255) so
                    # the f32->uint8 saturating conversion implements the
                    # clip (saturate at 255 == min at 1.0 pre-scale); the
                    # host applies the final min(., hi).  Halves out-DMA
                    # bytes and removes the DVE stage from the chain.
                    o_t = op.tile([128, CW], mybir.dt.uint8, tag="o")
                    nc.scalar.activation(
                        out=o_t[:, :w],
                        in_=ps[:, :w],
                        func=mybir.ActivationFunctionType.Exp,
                        scale=float(cfg.inv_scale),
                        bias=b_sb[:, pair : pair + 1],
                    )
                    pending_outs.append((o_t, pair, c0, w))

            # all out-DMAs dispatch after the input loop: by now the SP
            # queue has no pending tails loads, so each out's sem-wait can't
            # head-of-line block an input, and they drain in production order
            for o_t, pair, c0, w in pending_outs:
                nc.sync.dma_start(
                    out=out[pair * 128 : (pair + 1) * 128, c0 : c0 + w],
                    in_=o_t[:, :w],
                )

    nc.compile()
    return nc


def _pow2_scale(maxabs: float, target: float = 120.0) -> float:
    if not (maxabs > 0):
        return 1.0
    return 2.0 ** math.floor(math.log2(target / maxabs))


def _prepare(cfg_base, ent_emb, rel_emb, head_ent_vec, obs_idx, obs_mask, rel_id,
             num_heads, train_mask):
    """Host-side prep. Returns (cfg, in_maps, host) where host carries what
    the epilogue on the host needs (m/cnt masks, fallback info)."""
    ent_emb = np.asarray(ent_emb, dtype=np.float32)
    rel_emb = np.asarray(rel_emb, dtype=np.float32)
    head_ent_vec = np.asarray(head_ent_vec, dtype=np.float32)
    obs_idx = np.asarray(obs_idx, dtype=np.int32)
    obs_mask = np.asarray(obs_mask, bool)
    rel_id = int(rel_id)
    num_heads = int(num_heads)
    train_mask = int(train_mask)

    D, H = cfg_base.d, cfg_base.h
    E_SH, N_CORES = cfg_base.e_sh, cfg_base.n_cores
    R = D // 2
    assert ent_emb.shape == (cfg_base.n_ent, D)
    assert num_heads == H
    assert obs_idx.shape == (H, obs_idx.shape[1])

    heads = np.flatnonzero(head_ent_vec != 0.0)
    assert heads.size == H, f"expected {H} heads, got {heads.size}"

    h = ent_emb[heads]
    r = rel_emb[rel_id]
    q = np.concatenate(
        [
            h[:, :R] * r[:R] - h[:, R:] * r[R:],
            h[:, :R] * r[R:] + h[:, R:] * r[:R],
        ],
        axis=1,
    ).astype(np.float32)

    cq = _pow2_scale(float(np.abs(q).max()))
    ct = _pow2_scale(float(np.abs(ent_emb).max()))
    inv_scale = 1.0 / (cq * ct)

    q8 = (q * cq).astype(FP8_NP)
    t8 = (ent_emb * ct).astype(FP8_NP)
    q8f = q8.astype(np.float32)

    # Observed-tail scores from the same quantized values the device uses.
    tg = t8[obs_idx].astype(np.float32)  # [H, MAX_OBS, D]
    s_obs = np.einsum("hd,hkd->hk", q8f, tg, optimize=True) * inv_scale
    mk = obs_mask.astype(np.float64)
    cnt = mk.sum(axis=1)
    Dh = (np.exp(s_obs.astype(np.float64)) * mk).sum(axis=1)
    m = np.where(cnt > 0, cnt / np.maximum(Dh, 1e-300), 0.0)
    # +ln(255): the device activation emits exp(s + ln m + ln 255) as uint8,
    # saturating at 255; the host divides by 255 and clips at hi
    lnm = np.where(
        cnt > 0, np.log(np.maximum(m, 1e-300)) + np.log(255.0), -30.0
    ).astype(np.float32)

    bias_np = np.ascontiguousarray(lnm.reshape(2, 128).T)  # [128, 2]

    qT_np = np.ascontiguousarray(
        q8.T.reshape(cfg_base.n_k, 128, H).transpose(1, 0, 2).reshape(128, -1)
    )

    hi = 1.0 - EPSILON if train_mask else 1.0
    cfg = Cfg(
        n_cores=N_CORES,
        n_ent=cfg_base.n_ent,
        d=D,
        h=H,
        cw=cfg_base.cw,
        head_w=cfg_base.head_w,
        hi=hi,
        inv_scale=inv_scale,
    )

    rem_w = cfg.chunks[0][1]
    main_w = E_SH - rem_w
    q_u8 = qT_np.view(np.uint8)
    bias_u8 = bias_np.astype("<f4").view(np.uint8)
    in_maps = []
    for c in range(N_CORES):
        sh = t8[c * E_SH : (c + 1) * E_SH]  # [E_SH, D]
        lay = sh.T.reshape(cfg.n_k, 128, E_SH).transpose(1, 0, 2)  # [128, k, e]
        tailsT_np = np.ascontiguousarray(lay[:, :, :main_w].reshape(128, -1))
        rem_u8 = np.ascontiguousarray(
            lay[:, :, main_w:].reshape(128, -1)
        ).view(np.uint8)
        head_np = np.ascontiguousarray(
            np.concatenate([q_u8, bias_u8, rem_u8], axis=1)
        )
        in_maps.append({"tailsT": tailsT_np, "headT": head_np})

    # ---- safety margins for the algebraic shortcuts ----
    # |s_h(t)| <= ||q8_h|| * max_t ||t8_t|| * inv_scale  (Cauchy-Schwarz)
    qn = np.linalg.norm(q8f, axis=1)
    tn_max = float(np.sqrt(np.einsum("td,td->t", t8.astype(np.float32),
                                     t8.astype(np.float32)).max()))
    smax_h = qn * tn_max * inv_scale
    # no-obs rows are all-zero iff max prob < THRESHOLD:
    #   prob_max <= exp(2*smax_h) / n_ent
    ok_zero = np.exp(2.0 * smax_h) / cfg.n_ent < THRESHOLD * 0.8
    # obs rows never hit the threshold iff min scaled > THRESHOLD:
    #   scaled_min >= exp(-2*smax_h)
    ok_obs = np.exp(-2.0 * smax_h) > THRESHOLD * 10.0
    need_exact = ((cnt == 0) & ~ok_zero) | ((cnt > 0) & ~ok_obs)

    host = {
        "cnt": cnt,
        "m": m,
        "obs_idx": obs_idx,
        "obs_mask": obs_mask,
        "train_mask": train_mask,
        "hi": hi,
        "need_exact": need_exact,
        "q8f": q8f,
        "t8": t8,
        "inv_scale": inv_scale,
    }
    return cfg, in_maps, host


def _exact_rows(host, rows):
    """Reference math for a handful of heads, from the quantized scores the
    device would have produced (fallback when a margin check fails)."""
    q8f = host["q8f"]
    t8f = host["t8"].astype(np.float32)
    s = (q8f[rows] @ t8f.T).astype(np.float64) * host["inv_scale"]  # [r, N]
    e = np.exp(s - s.max(axis=1, keepdims=True))
    prob = e / e.sum(axis=1, keepdims=True)
    mk = host["obs_mask"][rows].astype(np.float64)
    obs = host["obs_idx"][rows]
    denom = (np.take_along_axis(prob, obs, axis=1) * mk).sum(axis=1)
    cnt = mk.sum(axis=1)
    scal = np.where(cnt > 0, cnt / np.maximum(denom, 1e-300), 1.0)
    scaled = prob * scal[:, None]
    sparse = np.where(scaled > THRESHOLD, scaled, 0.0)
    clamped = np.clip(sparse, 0.0, host["hi"])
    if host["train_mask"]:
        rr = np.arange(len(rows))[:, None]
        sel = (mk > 0)
        clamped[np.broadcast_to(rr, obs.shape)[sel], obs[sel]] = 1.0
    return clamped.astype(np.float32)


def kernel(ent_emb, rel_emb, head_ent_vec, obs_idx, obs_mask, rel_id, num_heads,
           train_mask):
    cfg, in_maps, host = _prepare(
        Cfg(), ent_emb, rel_emb, head_ent_vec, obs_idx, obs_mask, rel_id,
        num_heads, train_mask,
    )
    if cfg not in _compile_cache:
        _compile_cache[cfg] = _build(cfg)
    nc = _compile_cache[cfg]
    res = None
    for attempt in range(3):
        try:
            res = run_bass_kernel_spmd(
                nc, in_maps, core_ids=list(range(cfg.n_cores))
            )
            break
        except Exception:
            # transient NRT device errors (NRT_EXEC_UNIT_UNRECOVERABLE) show
            # up occasionally on back-to-back runs; back off and retry
            if attempt == 2:
                raise
            import time as _time

            _time.sleep(5.0 * (attempt + 1))
    out = np.concatenate(
        [np.asarray(res.results[c]["out"]) for c in range(cfg.n_cores)], axis=1
    ).astype(np.float32)
    out *= np.float32(1.0 / 255.0)
    np.minimum(out, np.float32(host["hi"]), out=out)

    cnt = host["cnt"]
    # no-obs heads: provably all below the sparsity threshold -> zero rows
    out[cnt == 0] = 0.0
    if host["train_mask"]:
        rows = np.arange(cfg.h)[:, None]
        sel = host["obs_mask"]
        out[np.broadcast_to(rows, host["obs_idx"].shape)[sel],
            host["obs_idx"][sel]] = 1.0
    if host["need_exact"].any():
        rows = np.flatnonzero(host["need_exact"])
        out[rows] = _exact_rows(host, rows)
    return out

